# revision 29
# baseline (speedup 1.0000x reference)
"""GATConv edge-parallel Bass kernel for TRN2 (8 NeuronCores).

Strategy (v2):
- Host: fold weights (rotation + d-major permutation), shard edges by dst
  range across 8 cores, sort by dst block, split by src table-half (int16
  indexing), build per-core index/meta tensors. Index-only preprocessing.
- Device (per core, SPMD):
  P0:  table build  tbl[n, j] = h_aug[n] @ W2cat[:, j] (fp16 matmuls, table
       fp16 in DRAM).
  P0b: per-node dst-score for the core's own node range (sd16, SBUF).
  P1.5: sdrep build — per block transpose sd16 then PE ones-broadcast so
       every partition holds the block's 128x4 dst scores along the free dim.
  P2:  per dst block: dma_gather src rows (runtime counts, 4 SWDGE queues),
       batched one-hot build on DVE (one is_equal for all tiles), per-edge
       dst-score via DVE mul+reduce against sdrep, lrelu+exp on ACT written
       straight into the fused [val||w] staging buffer, single fused
       scatter+z matmul per tile into PSUM, normalize + unrotate, write out.
"""
import numpy as np

import concourse.bass as bass
import concourse.bacc as bacc
import concourse.mybir as mybir
import concourse.tile as tile
from concourse.bass_utils import run_bass_kernel_spmd

F16 = mybir.dt.float16
F32 = mybir.dt.float32
I16 = mybir.dt.int16
I32 = mybir.dt.int32
AX = mybir.AxisListType
OP = mybir.AluOpType
AF = mybir.ActivationFunctionType


class Cfg:
    def __init__(self, N, E, NC=8, IN=32, D=32, H=4, VOFF=25088, neg=0.01):
        assert N % NC == 0
        self.N, self.E, self.NC, self.IN, self.D, self.H = N, E, NC, IN, D, H
        self.F = D * H                     # 128 feature cols
        self.NPC = N // NC                 # nodes per core
        self.NBLK = (self.NPC + 127) // 128
        self.NTBL = (N + 127) // 128       # table blocks (node projection)
        self.VOFF = min(VOFF, N)           # table split for int16 indexing
        self.neg = neg
        self.TCH = 16                      # P0 table-build blocks per DMA chunk


def fold_weights(cfg, W_lin, b_lin, W_att, b_att, seed=1234):
    D, H, IN = cfg.D, cfg.H, cfg.IN
    rng = np.random.default_rng(seed)
    Wa_s, Wa_d = W_att[:, :D].astype(np.float64), W_att[:, D:].astype(np.float64)
    W_lin = W_lin.astype(np.float64)
    b_lin = b_lin.astype(np.float64)
    Rs, Rinvs = [], []
    for hh in range(H):
        a = Wa_s[hh]
        Q, _ = np.linalg.qr(
            np.concatenate([a[:, None], rng.standard_normal((D, D - 1))], axis=1)
        )
        R = Q.T.copy()
        R[0] = a
        Rs.append(R)
        Rinvs.append(np.linalg.inv(R))
    W2 = np.stack([Rs[hh] @ W_lin[hh * D:(hh + 1) * D] for hh in range(H)])
    b2 = np.stack([Rs[hh] @ b_lin[hh * D:(hh + 1) * D] for hh in range(H)])
    Wt = np.zeros((IN, H * D)); bt = np.zeros(H * D)
    for hh in range(H):
        for d in range(D):
            Wt[:, d * H + hh] = W2[hh, d]
            bt[d * H + hh] = b2[hh, d]
    Ud = np.zeros((IN, H)); cd = np.zeros(H)
    for hh in range(H):
        Ud[:, hh] = W_lin[hh * D:(hh + 1) * D].T @ Wa_d[hh]
        cd[hh] = b_lin[hh * D:(hh + 1) * D] @ Wa_d[hh] + b_att[hh]
    W2cat = np.concatenate(
        [np.concatenate([Wt, Ud], 1), np.concatenate([bt, cd])[None, :]], 0
    ).astype(np.float32)                                   # [IN+1, F+H]
    RinvP = np.zeros((H * D, H * D))
    for hh in range(H):
        Rin = Rinvs[hh]
        for d in range(D):
            for dp in range(D):
                RinvP[d * H + hh, dp * H + hh] = Rin[dp, d]
    return W2cat, RinvP.astype(np.float32)


def wrap16(iv, ni):
    """int16 index list [ni] -> [128, ni//16] wrapped-16, replicated x8."""
    assert len(iv) == ni and ni % 16 == 0
    out = np.zeros((128, ni // 16), np.int16)
    blk = iv.reshape(ni // 16, 16).T
    for g in range(8):
        out[g * 16:(g + 1) * 16] = blk
    return out


def prep_edges(cfg, src, dst):
    """Per-core index prep. Returns (T_L, T_H, per_core list of dicts)."""
    src = np.asarray(src).astype(np.int64)
    dst = np.asarray(dst).astype(np.int64)
    cores = []
    # first pass: collect per (core, block, half) index lists, find maxima
    all_groups = []
    T_L = T_H = 0
    for c in range(cfg.NC):
        n0 = c * cfg.NPC
        m = (dst >= n0) & (dst < n0 + cfg.NPC)
        s, d = src[m], dst[m] - n0
        order = np.argsort(d, kind="stable")
        s, d = s[order], d[order]
        groups = []
        for b in range(cfg.NBLK):
            bm = (d >= b * 128) & (d < (b + 1) * 128)
            sb, db = s[bm], d[bm] - b * 128
            lo = sb < cfg.VOFF
            gl = (sb[lo], db[lo])
            gh = (sb[~lo] - cfg.VOFF, db[~lo])
            groups.append((gl, gh))
            T_L = max(T_L, -(-max(len(gl[0]), 1) // 128))
            T_H = max(T_H, -(-max(len(gh[0]), 1) // 128))
        all_groups.append(groups)
    if cfg.N <= cfg.VOFF:
        T_H = 0
    NI_L, NI_H = T_L * 128, T_H * 128
    T_B = T_L + T_H
    for c in range(cfg.NC):
        idxL = np.zeros((128, cfg.NBLK * max(NI_L, 16) // 16), np.int16)
        idxH = np.zeros((128, cfg.NBLK * max(NI_H, 16) // 16), np.int16) if T_H else None
        dstloc = np.full((128, cfg.NBLK * T_B), 999.0, np.float16)
        dstrep = np.full((128, cfg.NBLK * T_B * 128), 999.0, np.float16)
        cnts = np.zeros((1, 4 * cfg.NBLK), np.int32)
        for b in range(cfg.NBLK):
            (sl, dl), (sh, dh) = all_groups[c][b]
            for half, (sv, dv, ni, t_off) in enumerate(
                [(sl, dl, NI_L, 0), (sh, dh, NI_H, T_L)]
            ):
                if ni == 0:
                    continue
                n = len(sv)
                iv = np.full(ni, -1, np.int16)
                iv[:n] = sv.astype(np.int16)
                # split into two half-gathers at a 128-multiple boundary
                nh1 = (ni // 2 + 127) // 128 * 128
                n1, n2 = min(n, nh1), max(n - nh1, 0)
                if n1 == 0:
                    iv[0], n1 = 0, 1
                if n2 == 0 and nh1 < ni:
                    iv[nh1], n2 = 0, 1
                tgt = idxL if half == 0 else idxH
                tgt[:, b * ni // 16:(b + 1) * ni // 16] = wrap16(iv, ni)
                cnts[0, 4 * b + 2 * half] = n1
                cnts[0, 4 * b + 2 * half + 1] = n2
                # dst_local placement: edge i -> (p=i%128, tile=t_off+i//128)
                i = np.arange(n)
                p, t = i % 128, t_off + i // 128
                dstloc[p, b * T_B + t] = dv.astype(np.float16)
                dstrep[:, (b * T_B + t) * 128 + p] = dv.astype(np.float16)[None, :]
        cores.append(dict(idxL=idxL, idxH=idxH, dstloc=dstloc, dstrep=dstrep,
                          cnts=cnts))
    return T_L, T_H, cores


def build_bass(cfg, T_L, T_H):
    T_B = T_L + T_H
    NI_L, NI_H = T_L * 128, T_H * 128
    F, FH = cfg.F, cfg.F + cfg.H          # 128, 132
    H = cfg.H
    INA = cfg.IN + 1                      # 33 (augmented with ones row)
    NQ = 4

    nc = bacc.Bacc("TRN2", num_devices=cfg.NC, debug=False,
                   dynamic_dma_scratch_size=65536, num_swdge_queues=NQ)

    hT = nc.dram_tensor("hT", [INA, cfg.N], F16, kind="ExternalInput")
    W2cat = nc.dram_tensor("W2cat", [INA, FH], F16, kind="ExternalInput")
    Rinv = nc.dram_tensor("Rinv", [F, F], F16, kind="ExternalInput")
    iRcm = nc.dram_tensor("iRcm", [128, 128 * T_B], F16, kind="ExternalInput")
    iCbig = nc.dram_tensor("iCbig", [128, T_B * 128], F16, kind="ExternalInput")
    ident = nc.dram_tensor("ident", [128, 128], F16, kind="ExternalInput")
    idxL = nc.dram_tensor("idxL", [128, cfg.NBLK * max(NI_L, 16) // 16], I16,
                          kind="ExternalInput")
    if T_H:
        idxH = nc.dram_tensor("idxH", [128, cfg.NBLK * NI_H // 16], I16,
                              kind="ExternalInput")
    dstloc = nc.dram_tensor("dstloc", [128, cfg.NBLK * T_B], F16,
                            kind="ExternalInput")
    dstrep = nc.dram_tensor("dstrep", [128, cfg.NBLK * T_B * 128], F16,
                            kind="ExternalInput")
    cnts = nc.dram_tensor("cnts", [1, 4 * cfg.NBLK], I32, kind="ExternalInput")
    hTown = nc.dram_tensor("hTown", [INA, cfg.NBLK * 128], F16,
                           kind="ExternalInput")
    out = nc.dram_tensor("out", [cfg.NBLK * 128, F], F32, kind="ExternalOutput")
    VO = cfg.VOFF
    tblL = nc.dram_tensor("tblL", [VO, F], F16, kind="Internal")
    tblH = (nc.dram_tensor("tblH", [cfg.NTBL * 128 - VO, F], F16, kind="Internal")
            if cfg.NTBL * 128 > VO else None)

    with tile.TileContext(nc) as tc:
        import contextlib
        with contextlib.ExitStack() as ctx:
            cpool = ctx.enter_context(tc.tile_pool(name="consts", bufs=1))
            p0pool = ctx.enter_context(tc.tile_pool(name="p0", bufs=2))
            stpool = ctx.enter_context(tc.tile_pool(name="stage", bufs=6))
            ohpool = ctx.enter_context(tc.tile_pool(name="oh", bufs=3))
            vpool = ctx.enter_context(tc.tile_pool(name="vp", bufs=3))
            scpool = ctx.enter_context(tc.tile_pool(name="sc", bufs=2))
            fpool = ctx.enter_context(tc.tile_pool(name="fin", bufs=2))
            blkps = ctx.enter_context(tc.tile_pool(name="blkps", bufs=3, space="PSUM"))
            sdeps = ctx.enter_context(tc.tile_pool(name="sdeps", bufs=2, space="PSUM"))
            finps = ctx.enter_context(tc.tile_pool(name="finps", bufs=2, space="PSUM"))

            # ---- constants ----
            W2c = cpool.tile([INA, FH], F16)
            nc.sync.dma_start(out=W2c[:], in_=W2cat[:])
            Rinv16 = cpool.tile([F, F], F16)
            nc.sync.dma_start(out=Rinv16[:], in_=Rinv[:])
            iRc = cpool.tile([128, 128 * T_B], F16)
            nc.sync.dma_start(out=iRc[:], in_=iRcm[:])
            iCb = cpool.tile([128, T_B * 128], F16)
            nc.sync.dma_start(out=iCb[:], in_=iCbig[:])
            idn = cpool.tile([128, 128], F16)
            nc.sync.dma_start(out=idn[:], in_=ident[:])
            idxLt = cpool.tile(list(idxL.shape), I16)
            nc.sync.dma_start(out=idxLt[:], in_=idxL[:])
            if T_H:
                idxHt = cpool.tile(list(idxH.shape), I16)
                nc.sync.dma_start(out=idxHt[:], in_=idxH[:])
            dloc = cpool.tile(list(dstloc.shape), F16)
            nc.sync.dma_start(out=dloc[:], in_=dstloc[:])
            cnt = cpool.tile([1, 4 * cfg.NBLK], I32)
            nc.sync.dma_start(out=cnt[:], in_=cnts[:])
            sd16 = cpool.tile([128, cfg.NBLK * H], F16)

            # ---- P0: table build (fp16) ----
            nch = cfg.NTBL // cfg.TCH + (1 if cfg.NTBL % cfg.TCH else 0)
            for ch in range(nch):
                b0 = ch * cfg.TCH
                nb = min(cfg.TCH, cfg.NTBL - b0)
                hchunk = p0pool.tile([INA, cfg.TCH * 128], F16, tag="hch")
                n_lo = b0 * 128
                n_hi = min(cfg.N, (b0 + nb) * 128)
                nc.sync.dma_start(out=hchunk[:, :n_hi - n_lo],
                                  in_=hT[:, n_lo:n_hi])
                rows = p0pool.tile([128, cfg.TCH * F], F16, tag="rows")
                for j0 in range(0, nb, 3):
                    jn = min(3, nb - j0)
                    ps = blkps.tile([128, 3 * FH], F32, tag="pblk")
                    for j in range(j0, j0 + jn):
                        nc.tensor.matmul(
                            ps[:, (j - j0) * FH:(j - j0) * FH + FH],
                            lhsT=hchunk[:, j * 128:(j + 1) * 128],
                            rhs=W2c[:], start=True, stop=True)
                    dst_ap = rows[:, j0 * F:(j0 + jn) * F
                                  ].rearrange("p (j f) -> p j f", f=F)
                    src_ap = ps[:, :jn * FH].rearrange(
                        "p (j f) -> p j f", f=FH)[:, :, :F]
                    if (j0 // 3) % 3:
                        nc.scalar.copy(dst_ap, src_ap)
                    else:
                        nc.vector.tensor_copy(dst_ap, src_ap)
                r0, r1 = b0 * 128, (b0 + nb) * 128
                if r1 <= VO or tblH is None:
                    nc.sync.dma_start(
                        out=tblL[r0:r1, :].rearrange("(j p) d -> p j d", p=128),
                        in_=rows[:, :nb * F].rearrange("p (j d) -> p j d", d=F))
                elif r0 >= VO:
                    nc.sync.dma_start(
                        out=tblH[r0 - VO:r1 - VO, :].rearrange(
                            "(j p) d -> p j d", p=128),
                        in_=rows[:, :nb * F].rearrange("p (j d) -> p j d", d=F))
                else:
                    nbl = (VO - r0) // 128
                    nc.sync.dma_start(
                        out=tblL[r0:VO, :].rearrange("(j p) d -> p j d", p=128),
                        in_=rows[:, :nbl * F].rearrange("p (j d) -> p j d", d=F))
                    nc.sync.dma_start(
                        out=tblH[0:r1 - VO, :].rearrange("(j p) d -> p j d", p=128),
                        in_=rows[:, nbl * F:nb * F].rearrange(
                            "p (j d) -> p j d", d=F))

            # ---- P0b: own-range sd (per-core hTown input) ----
            for b in range(cfg.NBLK):
                hch = p0pool.tile([INA, 128], F16, tag="hown")
                nc.sync.dma_start(out=hch[:], in_=hTown[:, b * 128:(b + 1) * 128])
                ps = sdeps.tile([128, H], F32, tag="sde")
                nc.tensor.matmul(ps[:], lhsT=hch[:], rhs=W2c[:, F:FH],
                                 start=True, stop=True)
                nc.scalar.copy(sd16[:, b * H:(b + 1) * H], ps[:])

            # ---- P2: main edge loop ----
            reg0 = nc.gpsimd.alloc_register()
            reg1 = nc.gpsimd.alloc_register()
            reg2 = nc.gpsimd.alloc_register()
            reg3 = nc.gpsimd.alloc_register()
            regs = [reg0, reg1, reg2, reg3]
            NI_L1 = (NI_L // 2 + 127) // 128 * 128
            NI_H1 = (NI_H // 2 + 127) // 128 * 128 if T_H else 0
            for b in range(cfg.NBLK):
                stg = stpool.tile([128, T_B * 128], F16, tag="stage")
                if b < 6:  # must cover every stage-pool slot (bufs=6)
                    nc.gpsimd.memset(stg[:], 0.0)
                drep = vpool.tile([128, T_B * 128], F16, tag="drep")
                nc.scalar.dma_start(
                    out=drep[:],
                    in_=dstrep[:, b * T_B * 128:(b + 1) * T_B * 128])
                for gi, (tb, n0, n1, ioff) in enumerate(
                    [(tblL, 0, NI_L1, 0), (tblL, NI_L1, NI_L, 0)] +
                    ([(tblH, NI_L, NI_L + NI_H1, 1),
                      (tblH, NI_L + NI_H1, NI_L + NI_H, 1)] if T_H else [])
                ):
                    ibase = (b * NI_L // 16 if ioff == 0
                             else b * NI_H // 16)
                    i0 = ibase + (n0 - (0 if ioff == 0 else NI_L)) // 16
                    i1 = ibase + (n1 - (0 if ioff == 0 else NI_L)) // 16
                    idxt = idxLt if ioff == 0 else idxHt
                    nc.gpsimd.reg_load(regs[gi], cnt[0:1, 4 * b + gi:4 * b + gi + 1])
                    nc.gpsimd.dma_gather(
                        stg[:, n0:n1].rearrange("p (k d) -> p k d", d=128),
                        tb[:, :], idxt[:, i0:i1],
                        n1 - n0, regs[gi], F, single_packet=False,
                        queue_num=(gi + b) % 4)

                # one-hot build (c-major, fully dense inner steps -> 2x)
                oh = ohpool.tile([128, 128 * T_B], F16, tag="oh")
                nc.vector.tensor_tensor(
                    out=oh[:].rearrange("p (c t) -> p c t", t=T_B),
                    in0=dloc[:, b * T_B:(b + 1) * T_B
                             ].rearrange("p (one t) -> p one t", one=1
                                         ).to_broadcast([128, 128, T_B]),
                    in1=iRc[:].rearrange("p (c t) -> p c t", t=T_B),
                    op=OP.is_equal)

                # oT: one-hot transposed, dense tensor_tensor vs per-
                # partition iota constant (2x mode)
                oT = ohpool.tile([128, T_B * 128], F16, tag="oT")
                nc.vector.tensor_tensor(
                    out=oT[:], in0=drep[:], in1=iCb[:], op=OP.is_equal)
                # per-edge dst score via PE: sde[slot, h] = oT.T-style matmuls
                sde32 = finps.tile([128, T_B * H], F32, tag="fin")
                for t in range(T_B):
                    nc.tensor.matmul(sde32[:, t * H:(t + 1) * H],
                                     lhsT=oT[:, t * 128:(t + 1) * 128],
                                     rhs=sd16[:, b * H:(b + 1) * H],
                                     start=True, stop=True)
                # sc = sde + s_src (first H cols of each gathered tile)
                sc16 = scpool.tile([128, T_B * H], F16, tag="sc16")
                nc.vector.tensor_add(
                    sc16[:].rearrange("p (t h) -> p t h", h=H),
                    sde32[:].rearrange("p (t h) -> p t h", h=H),
                    stg[:].rearrange("p (t d) -> p t d", d=128)[:, :, :H])
                # w = exp(leaky_relu(sc)) = max(exp(sc), exp(neg*sc))
                e1 = scpool.tile([128, T_B * H], F16, tag="e1")
                nc.scalar.activation(e1[:], sc16[:], AF.Exp)
                e2 = scpool.tile([128, T_B * H], F16, tag="e2")
                nc.scalar.activation(e2[:], sc16[:], AF.Exp, scale=cfg.neg)
                w16 = scpool.tile([128, T_B * H], F16, tag="w16")
                nc.vector.tensor_max(w16[:], e1[:], e2[:])
                # val_cat: per-tile 144-col stride, [val 128 | w 4 | pad 12]
                vcat = vpool.tile([128, T_B * 256], F16, tag="vcat")
                vview = vcat[:].rearrange("p (t q h) -> p t q h", q=64, h=H)
                nc.scalar.copy(
                    vview[:, :, 32:33, :],
                    w16[:].rearrange("p (t one h) -> p t one h", one=1, h=H))
                # val = stg * w  (broadcast per 32-feature group)
                nc.vector.tensor_mul(
                    vview[:, :, 0:32, :],
                    stg[:].rearrange("p (t g h) -> p t g h", g=32, h=H),
                    w16[:].rearrange("p (t one h) -> p t one h", one=1, h=H
                                     ).to_broadcast([128, T_B, 32, H]))
                # fused scatter + z matmuls
                pblk = blkps.tile([128, FH], F32, tag="pblk")
                ohv = oh[:].rearrange("p (c t) -> p c t", t=T_B)
                for t in range(T_B):
                    nc.tensor.matmul(pblk[:], lhsT=ohv[:, :, t],
                                     rhs=vcat[:, t * 256:t * 256 + 132],
                                     start=(t == 0), stop=(t == T_B - 1))
                # finalize
                zc = fpool.tile([128, H], F32, tag="zc")
                nc.vector.tensor_scalar_max(zc[:], pblk[:, F:FH], 1e-30)
                rz = fpool.tile([128, H], F32, tag="rz")
                nc.vector.reciprocal(rz[:], zc[:])
                odiv = fpool.tile([128, F], F16, tag="odiv")
                nc.vector.tensor_mul(
                    odiv[:].rearrange("p (g h) -> p g h", g=32),
                    pblk[:, :F].rearrange("p (g h) -> p g h", g=32),
                    rz[:].rearrange("p (o h) -> p o h", o=1
                                    ).to_broadcast([128, 32, H]))
                oDp = finps.tile([128, 128], F16, tag="fin")
                nc.tensor.transpose(oDp[:], odiv[:], idn[:])
                odT = fpool.tile([128, F], F16, tag="odT")
                nc.scalar.copy(odT[:], oDp[:])
                finp = finps.tile([128, F], F32, tag="fin")
                nc.tensor.matmul(finp[:], lhsT=odT[:], rhs=Rinv16[:],
                                 start=True, stop=True)
                ofin = fpool.tile([128, F], F32, tag="ofin")
                nc.scalar.copy(ofin[:], finp[:])
                nc.sync.dma_start(out=out[b * 128:(b + 1) * 128, :], in_=ofin[:])
    nc.compile()
    return nc


def host_prep(cfg, h, W_lin, b_lin, W_att, b_att, src, dst):
    W2cat, RinvP = fold_weights(cfg, W_lin, b_lin, W_att, b_att)
    T_L, T_H, cores = prep_edges(cfg, src, dst)
    h_aug = np.concatenate(
        [np.asarray(h, np.float32), np.ones((cfg.N, 1), np.float32)], 1)
    hT = np.ascontiguousarray(h_aug.T).astype(np.float16)   # [33, N]
    T_B = T_L + T_H
    iRcm = np.repeat(np.arange(128, dtype=np.float16), T_B)[None, :].repeat(128, 0).copy()
    iCbig = np.broadcast_to(np.arange(128, dtype=np.float16)[:, None],
                            (128, T_B * 128)).copy()
    ident = np.eye(128, dtype=np.float16)
    common = dict(hT=hT, W2cat=W2cat.astype(np.float16),
                  Rinv=RinvP.astype(np.float16),
                  iRcm=iRcm, iCbig=iCbig, ident=ident)
    in_maps = []
    for c in range(cfg.NC):
        d = dict(common)
        cc = cores[c]
        d["idxL"] = cc["idxL"]
        if T_H:
            d["idxH"] = cc["idxH"]
        d["dstloc"] = cc["dstloc"]
        d["dstrep"] = cc["dstrep"]
        d["cnts"] = cc["cnts"]
        n0 = c * cfg.NPC
        own = np.zeros((cfg.IN + 1, cfg.NBLK * 128), np.float16)
        own[:, :cfg.NPC] = hT[:, n0:n0 + cfg.NPC]
        d["hTown"] = own
        in_maps.append(d)
    return T_L, T_H, in_maps


def run(cfg, inputs, trace=False):
    h, W_lin, b_lin = inputs["h"], inputs["W_lin"], inputs["b_lin"]
    W_att, b_att = inputs["W_att"], inputs["b_att"]
    src, dst = inputs["src"], inputs["dst"]
    T_L, T_H, in_maps = host_prep(cfg, h, W_lin, b_lin, W_att, b_att, src, dst)
    nc = build_bass(cfg, T_L, T_H)
    res = run_bass_kernel_spmd(nc, in_maps, core_ids=list(range(cfg.NC)),
                               trace=trace)
    outs = []
    for c in range(cfg.NC):
        outs.append(res.results[c]["out"][:cfg.NPC])      # [NPC, 128] d-major
    full = np.concatenate(outs, 0)                        # [N, 128]
    out = full.reshape(cfg.N, cfg.D, cfg.H).transpose(0, 2, 1)  # [N, H, D]
    return np.ascontiguousarray(out), res


# ---------------------------------------------------------------------------
# Harness entry point: kernel(**inputs) -> full output [50000, 4, 32] f32.
# Self-contained: shapes/sharding hardcoded for nn_GATConv (N=50000, E=800000,
# IN=32, OUT=32, H=4, 8 NeuronCores, edge-parallel by dst range).
# ---------------------------------------------------------------------------
_BUILD_CACHE = {}


def kernel(h, W_lin, b_lin, W_att, b_att, src, dst):
    h = np.asarray(h, np.float32)
    W_lin = np.asarray(W_lin, np.float32)
    b_lin = np.asarray(b_lin, np.float32)
    W_att = np.asarray(W_att, np.float32)
    b_att = np.asarray(b_att, np.float32)
    src = np.asarray(src).astype(np.int64)
    dst = np.asarray(dst).astype(np.int64)
    cfg = Cfg(h.shape[0], src.shape[0])
    T_L, T_H, in_maps = host_prep(cfg, h, W_lin, b_lin, W_att, b_att, src, dst)
    key = (cfg.N, cfg.E, T_L, T_H)
    if key not in _BUILD_CACHE:
        _BUILD_CACHE[key] = build_bass(cfg, T_L, T_H)
    nc = _BUILD_CACHE[key]
    res = run_bass_kernel_spmd(nc, in_maps, core_ids=list(range(cfg.NC)))
    outs = [res.results[c]["out"][:cfg.NPC] for c in range(cfg.NC)]
    full = np.concatenate(outs, 0)
    return np.ascontiguousarray(
        full.reshape(cfg.N, cfg.D, cfg.H).transpose(0, 2, 1)).astype(np.float32)


# revision 31
# speedup vs baseline: 1.0068x; 1.0068x over previous
"""GATConv edge-parallel Bass kernel for TRN2 (8 NeuronCores).

Strategy (v2):
- Host: fold weights (rotation + d-major permutation), shard edges by dst
  range across 8 cores, sort by dst block, split by src table-half (int16
  indexing), build per-core index/meta tensors. Index-only preprocessing.
- Device (per core, SPMD):
  P0:  table build  tbl[n, j] = h_aug[n] @ W2cat[:, j] (fp16 matmuls, table
       fp16 in DRAM).
  P0b: per-node dst-score for the core's own node range (sd16, SBUF).
  P1.5: sdrep build — per block transpose sd16 then PE ones-broadcast so
       every partition holds the block's 128x4 dst scores along the free dim.
  P2:  per dst block: dma_gather src rows (runtime counts, 4 SWDGE queues),
       batched one-hot build on DVE (one is_equal for all tiles), per-edge
       dst-score via DVE mul+reduce against sdrep, lrelu+exp on ACT written
       straight into the fused [val||w] staging buffer, single fused
       scatter+z matmul per tile into PSUM, normalize + unrotate, write out.
"""
import numpy as np

import concourse.bass as bass
import concourse.bacc as bacc
import concourse.mybir as mybir
import concourse.tile as tile
from concourse.bass_utils import run_bass_kernel_spmd

F16 = mybir.dt.float16
F32 = mybir.dt.float32
I16 = mybir.dt.int16
I32 = mybir.dt.int32
AX = mybir.AxisListType
OP = mybir.AluOpType
AF = mybir.ActivationFunctionType


class Cfg:
    def __init__(self, N, E, NC=8, IN=32, D=32, H=4, VOFF=25088, neg=0.01):
        assert N % NC == 0
        self.N, self.E, self.NC, self.IN, self.D, self.H = N, E, NC, IN, D, H
        self.F = D * H                     # 128 feature cols
        self.NPC = N // NC                 # nodes per core
        self.NBLK = (self.NPC + 127) // 128
        self.NTBL = (N + 127) // 128       # table blocks (node projection)
        self.VOFF = min(VOFF, N)           # table split for int16 indexing
        self.neg = neg
        self.TCH = 8                       # P0 table-build blocks per DMA chunk


def fold_weights(cfg, W_lin, b_lin, W_att, b_att, seed=1234):
    D, H, IN = cfg.D, cfg.H, cfg.IN
    rng = np.random.default_rng(seed)
    Wa_s, Wa_d = W_att[:, :D].astype(np.float64), W_att[:, D:].astype(np.float64)
    W_lin = W_lin.astype(np.float64)
    b_lin = b_lin.astype(np.float64)
    Rs, Rinvs = [], []
    for hh in range(H):
        a = Wa_s[hh]
        Q, _ = np.linalg.qr(
            np.concatenate([a[:, None], rng.standard_normal((D, D - 1))], axis=1)
        )
        R = Q.T.copy()
        R[0] = a
        Rs.append(R)
        Rinvs.append(np.linalg.inv(R))
    W2 = np.stack([Rs[hh] @ W_lin[hh * D:(hh + 1) * D] for hh in range(H)])
    b2 = np.stack([Rs[hh] @ b_lin[hh * D:(hh + 1) * D] for hh in range(H)])
    Wt = np.zeros((IN, H * D)); bt = np.zeros(H * D)
    for hh in range(H):
        for d in range(D):
            Wt[:, d * H + hh] = W2[hh, d]
            bt[d * H + hh] = b2[hh, d]
    Ud = np.zeros((IN, H)); cd = np.zeros(H)
    for hh in range(H):
        Ud[:, hh] = W_lin[hh * D:(hh + 1) * D].T @ Wa_d[hh]
        cd[hh] = b_lin[hh * D:(hh + 1) * D] @ Wa_d[hh] + b_att[hh]
    W2cat = np.concatenate(
        [np.concatenate([Wt, Ud], 1), np.concatenate([bt, cd])[None, :]], 0
    ).astype(np.float32)                                   # [IN+1, F+H]
    RinvP = np.zeros((H * D, H * D))
    for hh in range(H):
        Rin = Rinvs[hh]
        for d in range(D):
            for dp in range(D):
                RinvP[d * H + hh, dp * H + hh] = Rin[dp, d]
    return W2cat, RinvP.astype(np.float32)


def wrap16(iv, ni):
    """int16 index list [ni] -> [128, ni//16] wrapped-16, replicated x8."""
    assert len(iv) == ni and ni % 16 == 0
    out = np.zeros((128, ni // 16), np.int16)
    blk = iv.reshape(ni // 16, 16).T
    for g in range(8):
        out[g * 16:(g + 1) * 16] = blk
    return out


def prep_edges(cfg, src, dst):
    """Per-core index prep. Returns (T_L, T_H, per_core list of dicts)."""
    src = np.asarray(src).astype(np.int64)
    dst = np.asarray(dst).astype(np.int64)
    cores = []
    # first pass: collect per (core, block, half) index lists, find maxima
    all_groups = []
    T_L = T_H = 0
    for c in range(cfg.NC):
        n0 = c * cfg.NPC
        m = (dst >= n0) & (dst < n0 + cfg.NPC)
        s, d = src[m], dst[m] - n0
        order = np.argsort(d, kind="stable")
        s, d = s[order], d[order]
        groups = []
        for b in range(cfg.NBLK):
            bm = (d >= b * 128) & (d < (b + 1) * 128)
            sb, db = s[bm], d[bm] - b * 128
            lo = sb < cfg.VOFF
            gl = (sb[lo], db[lo])
            gh = (sb[~lo] - cfg.VOFF, db[~lo])
            groups.append((gl, gh))
            T_L = max(T_L, -(-max(len(gl[0]), 1) // 128))
            T_H = max(T_H, -(-max(len(gh[0]), 1) // 128))
        all_groups.append(groups)
    if cfg.N <= cfg.VOFF:
        T_H = 0
    NI_L, NI_H = T_L * 128, T_H * 128
    T_B = T_L + T_H
    for c in range(cfg.NC):
        idxL = np.zeros((128, cfg.NBLK * max(NI_L, 16) // 16), np.int16)
        idxH = np.zeros((128, cfg.NBLK * max(NI_H, 16) // 16), np.int16) if T_H else None
        dstloc = np.full((128, cfg.NBLK * T_B), 999.0, np.float16)
        dstrep = np.full((128, cfg.NBLK * T_B * 128), 999.0, np.float16)
        cnts = np.zeros((1, 4 * cfg.NBLK), np.int32)
        for b in range(cfg.NBLK):
            (sl, dl), (sh, dh) = all_groups[c][b]
            for half, (sv, dv, ni, t_off) in enumerate(
                [(sl, dl, NI_L, 0), (sh, dh, NI_H, T_L)]
            ):
                if ni == 0:
                    continue
                so = np.argsort(sv, kind="stable")
                sv, dv = sv[so], dv[so]
                n = len(sv)
                iv = np.full(ni, -1, np.int16)
                iv[:n] = sv.astype(np.int16)
                # split into two half-gathers at a 128-multiple boundary
                nh1 = (ni // 2 + 127) // 128 * 128
                n1, n2 = min(n, nh1), max(n - nh1, 0)
                if n1 == 0:
                    iv[0], n1 = 0, 1
                if n2 == 0 and nh1 < ni:
                    iv[nh1], n2 = 0, 1
                tgt = idxL if half == 0 else idxH
                tgt[:, b * ni // 16:(b + 1) * ni // 16] = wrap16(iv, ni)
                cnts[0, 4 * b + 2 * half] = n1
                cnts[0, 4 * b + 2 * half + 1] = n2
                # dst_local placement: edge i -> (p=i%128, tile=t_off+i//128)
                i = np.arange(n)
                p, t = i % 128, t_off + i // 128
                dstloc[p, b * T_B + t] = dv.astype(np.float16)
                dstrep[:, (b * T_B + t) * 128 + p] = dv.astype(np.float16)[None, :]
        cores.append(dict(idxL=idxL, idxH=idxH, dstloc=dstloc, dstrep=dstrep,
                          cnts=cnts))
    return T_L, T_H, cores


def build_bass(cfg, T_L, T_H):
    T_B = T_L + T_H
    NI_L, NI_H = T_L * 128, T_H * 128
    F, FH = cfg.F, cfg.F + cfg.H          # 128, 132
    H = cfg.H
    INA = cfg.IN + 1                      # 33 (augmented with ones row)
    NQ = 4

    nc = bacc.Bacc("TRN2", num_devices=cfg.NC, debug=False,
                   dynamic_dma_scratch_size=65536, num_swdge_queues=NQ)

    hT = nc.dram_tensor("hT", [INA, cfg.N], F16, kind="ExternalInput")
    W2cat = nc.dram_tensor("W2cat", [INA, FH], F16, kind="ExternalInput")
    Rinv = nc.dram_tensor("Rinv", [F, F], F16, kind="ExternalInput")
    iRcm = nc.dram_tensor("iRcm", [128, 128 * T_B], F16, kind="ExternalInput")
    iCbig = nc.dram_tensor("iCbig", [128, T_B * 128], F16, kind="ExternalInput")
    ident = nc.dram_tensor("ident", [128, 128], F16, kind="ExternalInput")
    idxL = nc.dram_tensor("idxL", [128, cfg.NBLK * max(NI_L, 16) // 16], I16,
                          kind="ExternalInput")
    if T_H:
        idxH = nc.dram_tensor("idxH", [128, cfg.NBLK * NI_H // 16], I16,
                              kind="ExternalInput")
    dstloc = nc.dram_tensor("dstloc", [128, cfg.NBLK * T_B], F16,
                            kind="ExternalInput")
    dstrep = nc.dram_tensor("dstrep", [128, cfg.NBLK * T_B * 128], F16,
                            kind="ExternalInput")
    cnts = nc.dram_tensor("cnts", [1, 4 * cfg.NBLK], I32, kind="ExternalInput")
    hTown = nc.dram_tensor("hTown", [INA, cfg.NBLK * 128], F16,
                           kind="ExternalInput")
    out = nc.dram_tensor("out", [cfg.NBLK * 128, F], F32, kind="ExternalOutput")
    VO = cfg.VOFF
    tblL = nc.dram_tensor("tblL", [VO, F], F16, kind="Internal")
    tblH = (nc.dram_tensor("tblH", [cfg.NTBL * 128 - VO, F], F16, kind="Internal")
            if cfg.NTBL * 128 > VO else None)

    with tile.TileContext(nc) as tc:
        import contextlib
        with contextlib.ExitStack() as ctx:
            cpool = ctx.enter_context(tc.tile_pool(name="consts", bufs=1))
            p0pool = ctx.enter_context(tc.tile_pool(name="p0", bufs=2))
            stpool = ctx.enter_context(tc.tile_pool(name="stage", bufs=6))
            ohpool = ctx.enter_context(tc.tile_pool(name="oh", bufs=3))
            vpool = ctx.enter_context(tc.tile_pool(name="vp", bufs=3))
            scpool = ctx.enter_context(tc.tile_pool(name="sc", bufs=2))
            fpool = ctx.enter_context(tc.tile_pool(name="fin", bufs=2))
            blkps = ctx.enter_context(tc.tile_pool(name="blkps", bufs=3, space="PSUM"))
            sdeps = ctx.enter_context(tc.tile_pool(name="sdeps", bufs=2, space="PSUM"))
            finps = ctx.enter_context(tc.tile_pool(name="finps", bufs=2, space="PSUM"))

            # ---- constants ----
            W2c = cpool.tile([INA, FH], F16)
            nc.sync.dma_start(out=W2c[:], in_=W2cat[:])
            Rinv16 = cpool.tile([F, F], F16)
            nc.sync.dma_start(out=Rinv16[:], in_=Rinv[:])
            iRc = cpool.tile([128, 128 * T_B], F16)
            nc.sync.dma_start(out=iRc[:], in_=iRcm[:])
            iCb = cpool.tile([128, T_B * 128], F16)
            nc.sync.dma_start(out=iCb[:], in_=iCbig[:])
            idn = cpool.tile([128, 128], F16)
            nc.sync.dma_start(out=idn[:], in_=ident[:])
            idxLt = cpool.tile(list(idxL.shape), I16)
            nc.sync.dma_start(out=idxLt[:], in_=idxL[:])
            if T_H:
                idxHt = cpool.tile(list(idxH.shape), I16)
                nc.sync.dma_start(out=idxHt[:], in_=idxH[:])
            dloc = cpool.tile(list(dstloc.shape), F16)
            nc.sync.dma_start(out=dloc[:], in_=dstloc[:])
            cnt = cpool.tile([1, 4 * cfg.NBLK], I32)
            nc.sync.dma_start(out=cnt[:], in_=cnts[:])
            sd16 = cpool.tile([128, cfg.NBLK * H], F16)

            # ---- P0: table build (fp16) ----
            nch = cfg.NTBL // cfg.TCH + (1 if cfg.NTBL % cfg.TCH else 0)
            for ch in range(nch):
                b0 = ch * cfg.TCH
                nb = min(cfg.TCH, cfg.NTBL - b0)
                hchunk = p0pool.tile([INA, cfg.TCH * 128], F16, tag="hch")
                n_lo = b0 * 128
                n_hi = min(cfg.N, (b0 + nb) * 128)
                nc.sync.dma_start(out=hchunk[:, :n_hi - n_lo],
                                  in_=hT[:, n_lo:n_hi])
                rows = p0pool.tile([128, cfg.TCH * F], F16, tag="rows")
                for j0 in range(0, nb, 3):
                    jn = min(3, nb - j0)
                    ps = blkps.tile([128, 3 * FH], F32, tag="pblk")
                    for j in range(j0, j0 + jn):
                        nc.tensor.matmul(
                            ps[:, (j - j0) * FH:(j - j0) * FH + FH],
                            lhsT=hchunk[:, j * 128:(j + 1) * 128],
                            rhs=W2c[:], start=True, stop=True)
                    dst_ap = rows[:, j0 * F:(j0 + jn) * F
                                  ].rearrange("p (j f) -> p j f", f=F)
                    src_ap = ps[:, :jn * FH].rearrange(
                        "p (j f) -> p j f", f=FH)[:, :, :F]
                    if (j0 // 3) % 3:
                        nc.scalar.copy(dst_ap, src_ap)
                    else:
                        nc.vector.tensor_copy(dst_ap, src_ap)
                r0, r1 = b0 * 128, (b0 + nb) * 128
                if r1 <= VO or tblH is None:
                    nc.sync.dma_start(
                        out=tblL[r0:r1, :].rearrange("(j p) d -> p j d", p=128),
                        in_=rows[:, :nb * F].rearrange("p (j d) -> p j d", d=F))
                elif r0 >= VO:
                    nc.sync.dma_start(
                        out=tblH[r0 - VO:r1 - VO, :].rearrange(
                            "(j p) d -> p j d", p=128),
                        in_=rows[:, :nb * F].rearrange("p (j d) -> p j d", d=F))
                else:
                    nbl = (VO - r0) // 128
                    nc.sync.dma_start(
                        out=tblL[r0:VO, :].rearrange("(j p) d -> p j d", p=128),
                        in_=rows[:, :nbl * F].rearrange("p (j d) -> p j d", d=F))
                    nc.sync.dma_start(
                        out=tblH[0:r1 - VO, :].rearrange("(j p) d -> p j d", p=128),
                        in_=rows[:, nbl * F:nb * F].rearrange(
                            "p (j d) -> p j d", d=F))

            # ---- P0b: own-range sd (per-core hTown input) ----
            for b in range(cfg.NBLK):
                hch = p0pool.tile([INA, 128], F16, tag="hown")
                nc.sync.dma_start(out=hch[:], in_=hTown[:, b * 128:(b + 1) * 128])
                ps = sdeps.tile([128, H], F32, tag="sde")
                nc.tensor.matmul(ps[:], lhsT=hch[:], rhs=W2c[:, F:FH],
                                 start=True, stop=True)
                nc.scalar.copy(sd16[:, b * H:(b + 1) * H], ps[:])

            # ---- P2: main edge loop ----
            reg0 = nc.gpsimd.alloc_register()
            reg1 = nc.gpsimd.alloc_register()
            reg2 = nc.gpsimd.alloc_register()
            reg3 = nc.gpsimd.alloc_register()
            regs = [reg0, reg1, reg2, reg3]
            NI_L1 = (NI_L // 2 + 127) // 128 * 128
            NI_H1 = (NI_H // 2 + 127) // 128 * 128 if T_H else 0
            for b in range(cfg.NBLK):
                stg = stpool.tile([128, T_B * 128], F16, tag="stage")
                if b < 6:  # must cover every stage-pool slot (bufs=6)
                    nc.gpsimd.memset(stg[:], 0.0)
                drep = vpool.tile([128, T_B * 128], F16, tag="drep")
                nc.sync.dma_start(
                    out=drep[:],
                    in_=dstrep[:, b * T_B * 128:(b + 1) * T_B * 128])
                for gi, (tb, n0, n1, ioff) in enumerate(
                    [(tblL, 0, NI_L1, 0), (tblL, NI_L1, NI_L, 0)] +
                    ([(tblH, NI_L, NI_L + NI_H1, 1),
                      (tblH, NI_L + NI_H1, NI_L + NI_H, 1)] if T_H else [])
                ):
                    ibase = (b * NI_L // 16 if ioff == 0
                             else b * NI_H // 16)
                    i0 = ibase + (n0 - (0 if ioff == 0 else NI_L)) // 16
                    i1 = ibase + (n1 - (0 if ioff == 0 else NI_L)) // 16
                    idxt = idxLt if ioff == 0 else idxHt
                    nc.gpsimd.reg_load(regs[gi], cnt[0:1, 4 * b + gi:4 * b + gi + 1])
                    nc.gpsimd.dma_gather(
                        stg[:, n0:n1].rearrange("p (k d) -> p k d", d=128),
                        tb[:, :], idxt[:, i0:i1],
                        n1 - n0, regs[gi], F, single_packet=False,
                        queue_num=(gi + b) % 4)

                # one-hot build (c-major, fully dense inner steps -> 2x)
                oh = ohpool.tile([128, 128 * T_B], F16, tag="oh")
                nc.vector.tensor_tensor(
                    out=oh[:].rearrange("p (c t) -> p c t", t=T_B),
                    in0=dloc[:, b * T_B:(b + 1) * T_B
                             ].rearrange("p (one t) -> p one t", one=1
                                         ).to_broadcast([128, 128, T_B]),
                    in1=iRc[:].rearrange("p (c t) -> p c t", t=T_B),
                    op=OP.is_equal)

                # oT: one-hot transposed, dense tensor_tensor vs per-
                # partition iota constant (2x mode)
                oT = ohpool.tile([128, T_B * 128], F16, tag="oT")
                nc.vector.tensor_tensor(
                    out=oT[:], in0=drep[:], in1=iCb[:], op=OP.is_equal)
                # per-edge dst score via PE: sde[slot, h] = oT.T-style matmuls
                sde32 = finps.tile([128, T_B * H], F32, tag="fin")
                for t in range(T_B):
                    nc.tensor.matmul(sde32[:, t * H:(t + 1) * H],
                                     lhsT=oT[:, t * 128:(t + 1) * 128],
                                     rhs=sd16[:, b * H:(b + 1) * H],
                                     start=True, stop=True)
                # sc = sde + s_src (first H cols of each gathered tile)
                sc16 = scpool.tile([128, T_B * H], F16, tag="sc16")
                nc.vector.tensor_add(
                    sc16[:].rearrange("p (t h) -> p t h", h=H),
                    sde32[:].rearrange("p (t h) -> p t h", h=H),
                    stg[:].rearrange("p (t d) -> p t d", d=128)[:, :, :H])
                # w = exp(leaky_relu(sc)) = max(exp(sc), exp(neg*sc))
                e1 = scpool.tile([128, T_B * H], F16, tag="e1")
                nc.scalar.activation(e1[:], sc16[:], AF.Exp)
                e2 = scpool.tile([128, T_B * H], F16, tag="e2")
                nc.scalar.activation(e2[:], sc16[:], AF.Exp, scale=cfg.neg)
                w16 = scpool.tile([128, T_B * H], F16, tag="w16")
                nc.vector.tensor_max(w16[:], e1[:], e2[:])
                # val_cat: per-tile 144-col stride, [val 128 | w 4 | pad 12]
                vcat = vpool.tile([128, T_B * 256], F16, tag="vcat")
                vview = vcat[:].rearrange("p (t q h) -> p t q h", q=64, h=H)
                nc.scalar.copy(
                    vview[:, :, 32:33, :],
                    w16[:].rearrange("p (t one h) -> p t one h", one=1, h=H))
                # val = stg * w  (broadcast per 32-feature group)
                nc.vector.tensor_mul(
                    vview[:, :, 0:32, :],
                    stg[:].rearrange("p (t g h) -> p t g h", g=32, h=H),
                    w16[:].rearrange("p (t one h) -> p t one h", one=1, h=H
                                     ).to_broadcast([128, T_B, 32, H]))
                # fused scatter + z matmuls
                pblk = blkps.tile([128, FH], F32, tag="pblk")
                ohv = oh[:].rearrange("p (c t) -> p c t", t=T_B)
                for t in range(T_B):
                    nc.tensor.matmul(pblk[:], lhsT=ohv[:, :, t],
                                     rhs=vcat[:, t * 256:t * 256 + 132],
                                     start=(t == 0), stop=(t == T_B - 1))
                # finalize
                zc = fpool.tile([128, H], F32, tag="zc")
                nc.vector.tensor_scalar_max(zc[:], pblk[:, F:FH], 1e-30)
                rz = fpool.tile([128, H], F32, tag="rz")
                nc.vector.reciprocal(rz[:], zc[:])
                odiv = fpool.tile([128, F], F16, tag="odiv")
                nc.vector.tensor_mul(
                    odiv[:].rearrange("p (g h) -> p g h", g=32),
                    pblk[:, :F].rearrange("p (g h) -> p g h", g=32),
                    rz[:].rearrange("p (o h) -> p o h", o=1
                                    ).to_broadcast([128, 32, H]))
                oDp = finps.tile([128, 128], F16, tag="fin")
                nc.tensor.transpose(oDp[:], odiv[:], idn[:])
                odT = fpool.tile([128, F], F16, tag="odT")
                nc.scalar.copy(odT[:], oDp[:])
                finp = finps.tile([128, F], F32, tag="fin")
                nc.tensor.matmul(finp[:], lhsT=odT[:], rhs=Rinv16[:],
                                 start=True, stop=True)
                ofin = fpool.tile([128, F], F32, tag="ofin")
                nc.scalar.copy(ofin[:], finp[:])
                nc.sync.dma_start(out=out[b * 128:(b + 1) * 128, :], in_=ofin[:])
    nc.compile()
    return nc


def host_prep(cfg, h, W_lin, b_lin, W_att, b_att, src, dst):
    W2cat, RinvP = fold_weights(cfg, W_lin, b_lin, W_att, b_att)
    T_L, T_H, cores = prep_edges(cfg, src, dst)
    h_aug = np.concatenate(
        [np.asarray(h, np.float32), np.ones((cfg.N, 1), np.float32)], 1)
    hT = np.ascontiguousarray(h_aug.T).astype(np.float16)   # [33, N]
    T_B = T_L + T_H
    iRcm = np.repeat(np.arange(128, dtype=np.float16), T_B)[None, :].repeat(128, 0).copy()
    iCbig = np.broadcast_to(np.arange(128, dtype=np.float16)[:, None],
                            (128, T_B * 128)).copy()
    ident = np.eye(128, dtype=np.float16)
    common = dict(hT=hT, W2cat=W2cat.astype(np.float16),
                  Rinv=RinvP.astype(np.float16),
                  iRcm=iRcm, iCbig=iCbig, ident=ident)
    in_maps = []
    for c in range(cfg.NC):
        d = dict(common)
        cc = cores[c]
        d["idxL"] = cc["idxL"]
        if T_H:
            d["idxH"] = cc["idxH"]
        d["dstloc"] = cc["dstloc"]
        d["dstrep"] = cc["dstrep"]
        d["cnts"] = cc["cnts"]
        n0 = c * cfg.NPC
        own = np.zeros((cfg.IN + 1, cfg.NBLK * 128), np.float16)
        own[:, :cfg.NPC] = hT[:, n0:n0 + cfg.NPC]
        d["hTown"] = own
        in_maps.append(d)
    return T_L, T_H, in_maps


def run(cfg, inputs, trace=False):
    h, W_lin, b_lin = inputs["h"], inputs["W_lin"], inputs["b_lin"]
    W_att, b_att = inputs["W_att"], inputs["b_att"]
    src, dst = inputs["src"], inputs["dst"]
    T_L, T_H, in_maps = host_prep(cfg, h, W_lin, b_lin, W_att, b_att, src, dst)
    nc = build_bass(cfg, T_L, T_H)
    res = run_bass_kernel_spmd(nc, in_maps, core_ids=list(range(cfg.NC)),
                               trace=trace)
    outs = []
    for c in range(cfg.NC):
        outs.append(res.results[c]["out"][:cfg.NPC])      # [NPC, 128] d-major
    full = np.concatenate(outs, 0)                        # [N, 128]
    out = full.reshape(cfg.N, cfg.D, cfg.H).transpose(0, 2, 1)  # [N, H, D]
    return np.ascontiguousarray(out), res


# ---------------------------------------------------------------------------
# Harness entry point: kernel(**inputs) -> full output [50000, 4, 32] f32.
# Self-contained: shapes/sharding hardcoded for nn_GATConv (N=50000, E=800000,
# IN=32, OUT=32, H=4, 8 NeuronCores, edge-parallel by dst range).
# ---------------------------------------------------------------------------
_BUILD_CACHE = {}


def kernel(h, W_lin, b_lin, W_att, b_att, src, dst):
    h = np.asarray(h, np.float32)
    W_lin = np.asarray(W_lin, np.float32)
    b_lin = np.asarray(b_lin, np.float32)
    W_att = np.asarray(W_att, np.float32)
    b_att = np.asarray(b_att, np.float32)
    src = np.asarray(src).astype(np.int64)
    dst = np.asarray(dst).astype(np.int64)
    cfg = Cfg(h.shape[0], src.shape[0])
    T_L, T_H, in_maps = host_prep(cfg, h, W_lin, b_lin, W_att, b_att, src, dst)
    key = (cfg.N, cfg.E, T_L, T_H)
    if key not in _BUILD_CACHE:
        _BUILD_CACHE[key] = build_bass(cfg, T_L, T_H)
    nc = _BUILD_CACHE[key]
    res = run_bass_kernel_spmd(nc, in_maps, core_ids=list(range(cfg.NC)))
    outs = [res.results[c]["out"][:cfg.NPC] for c in range(cfg.NC)]
    full = np.concatenate(outs, 0)
    return np.ascontiguousarray(
        full.reshape(cfg.N, cfg.D, cfg.H).transpose(0, 2, 1)).astype(np.float32)
